# revision 33
# baseline (speedup 1.0000x reference)
"""Self-contained Trainium2 Bass kernel for 16-head cross-attention MHA.

Problem: B=2, SQ=SK=2048, D=1024, H=16, key_size=64 (fp32 in/out).

Sharding (8 cores): data-parallel over batch (2) x tensor-parallel over
head groups (4 heads per core). Each core computes its 4 heads'
Q/K/V projections (column slices of wq/wk/wv), attention, and a partial
output projection (row slice of wo). Host sums the 4 partial outputs per
batch (bf16 partials, fp32 accumulate) and adds the (bv @ wo + bo)
correction (probs sum to 1, so bv contributes exactly bv @ wo; bk
cancels in softmax).

Per-core pipeline (bf16 matmuls, fp32 PSUM). The kernel is co-bound:
PE matmul ~200us busy, ScalarE exp effectively ~166us (128 EXP
instructions at (1024+284)/1.2GHz each plus ~190ns NX dispatch gap;
PSUM's 8 banks cap the exp tile at [128,1024] with 2-deep ping-pong).
The schedule keeps both engines dense:

  1. PE warm-up matmuls on a memset tile release the HAM clock gate
     (cold PE runs at 1.2 GHz for ~3.4us) while the first DMAs land.
  2. All inputs are host-pre-tiled so every DMA moves 4KB-contiguous
     lines, split into ~128KB partition-sliced pieces across the 16
     queues (2KB-burst DMA: small lines are descriptor-rate bound).
     Priority order: bq, wk, xeT keys 0-511, wq, wv, xdT m-half-0;
     the rest stream during phase 0. xdT halves are kept resident.
  3. Prefix computes only K^T keys 0-511, V' key tiles 0-3, and Q^T
     (both head pairs) for m 0-1023 -- the first score matmul issues
     after ~4.5MB instead of 10.5MB.
  4. Phase order (head_pair, m_half) = (0,0),(1,0),(0,1),(1,1).
     Scores^T tiles ([key_pos, query], contraction = head_dim) are
     exp'd by ScalarE from [128,1024] PSUM (scale=1/8 fused, no
     max-subtraction: scores ~ N(0,1)) into bf16 SBUF. Phase 0
     dribbles the remaining K0 chains just ahead of their scores,
     V' groups 2-7 (ones column carries the softmax denominator Z),
     and K1. ctx for phase i runs inside phase i+1 (1x jt 0-7, 2x jt
     8-11, normalization at jt 12) freeing the 4 ctx PSUM banks for
     the late Q-projection dribbles (m-half-1 in phases 1/2). Phase 3
     runs ctx(phase2) at 2x in its front half and its own ctx at 2x
     in the back half (start=True clears a whole PSUM bank, so every
     accumulation chain owns its banks).
  5. Normalization: U' evicted to SBUF, Z row partition-broadcast via
     GpSimd, DVE reciprocal + multiply into ctx^T bf16.
  6. Tail: out-projection mts 0-5 are emitted before the final norm
     (ctxT dependency tracking is coarse); evictions alternate
     ScalarE/DVE; partial output leaves as bf16.

Measured on 8 axon-tunneled trn2 cores: ~241us mean (239.7-242.4 over
4 runs), rel err 4.9e-3 (bf16-dominated; bf16 output adds ~0.1%).
"""

import os
import sys

for _p in ("/opt/trn_rl_repo", "/root/.axon_site/_ro/trn_rl_repo"):
    if os.path.isdir(_p) and _p not in sys.path:
        sys.path.insert(0, _p)

import numpy as np
import ml_dtypes

BF16 = ml_dtypes.bfloat16

B = 2
S = 2048          # SQ == SK
D = 1024
H = 16
KEY = 64
HPC = 4           # heads per core
NPC = HPC * KEY   # 256 per-core slice of D
KT = D // 128     # 8 contraction tiles for projections
NT = NPC // 128   # 2 head-pair tiles
MC = S // 512     # 4 m-chunks of 512
JT = S // 128     # 16 key tiles

_NC = None
LAST_RESULTS = None  # BassKernelResults of the most recent run (for test.py)


def _build_nc():
    import concourse.tile as tile
    from concourse import bacc, mybir

    FP32 = mybir.dt.float32
    BF = mybir.dt.bfloat16
    AF = mybir.ActivationFunctionType

    nc = bacc.Bacc("TRN2", target_bir_lowering=False, debug=False, num_devices=8)

    # Host pre-tiles every input so each DMA moves >=4KB-contiguous lines
    # (1KB lines are descriptor-rate bound: ~100ns per line per queue).
    xd_d = nc.dram_tensor("xd", [2, 128, KT, 1024], BF, kind="ExternalInput").ap()
    xe_d = nc.dram_tensor("xe", [4, 128, KT, 512], BF, kind="ExternalInput").ap()
    wq_d = nc.dram_tensor("wq", [128, KT, NPC], BF, kind="ExternalInput").ap()
    wk_d = nc.dram_tensor("wk", [128, KT, NPC], BF, kind="ExternalInput").ap()
    wv_d = nc.dram_tensor("wv", [128, KT, NPC], BF, kind="ExternalInput").ap()
    wo_d = nc.dram_tensor("wo", [128, NT, D], BF, kind="ExternalInput").ap()
    bq_d = nc.dram_tensor("bq", [NT, 128, 1], FP32, kind="ExternalInput").ap()
    o_d = nc.dram_tensor("o", [S, D], BF, kind="ExternalOutput").ap()

    with tile.TileContext(nc) as tc:
        with (
            tc.tile_pool(name="consts", bufs=1) as consts,
            tc.tile_pool(name="acts", bufs=1) as acts,
            tc.tile_pool(name="zp", bufs=2) as zp,
            tc.tile_pool(name="up", bufs=3) as up,
            tc.tile_pool(name="zbp", bufs=2) as zbp,
            tc.tile_pool(name="osb", bufs=4) as osb,
        ):
            # ---- PE warm-up (HAM clock gate): no-DMA dummy matmuls ----
            warm_sb = consts.tile([128, 512], BF, tag="warm")
            nc.vector.memset(warm_sb[:], 0.0)

            # ---- priority DMAs: partition-sliced pieces across queues ----
            wk_sb = consts.tile([128, KT, NPC], BF, tag="wk")
            wq_sb = consts.tile([128, KT, NPC], BF, tag="wq")
            wv_sb = consts.tile([128, KT, NPC], BF, tag="wv")
            xeT_sb = acts.tile([128, 4, KT, 512], BF, tag="xeT")
            xdh0_sb = acts.tile([128, KT, 1024], BF, tag="xdh0")
            xdh1_sb = acts.tile([128, KT, 1024], BF, tag="xdh1")

            def dma_w(dst, src, pieces, cols=slice(None)):
                ps_sz = 128 // pieces
                for i in range(pieces):
                    p0, p1 = i * ps_sz, (i + 1) * ps_sz
                    nc.sync.dma_start(dst[p0:p1, :, cols], src[p0:p1, :, cols])

            def dma_xe(mb):
                # 4KB lines: [32p, 4kt, 512] pieces, kt-half-major
                for kh in range(2):
                    for i in range(4):
                        p0, p1 = i * 32, (i + 1) * 32
                        nc.sync.dma_start(
                            xeT_sb[p0:p1, mb, kh * 4:(kh + 1) * 4, :],
                            xe_d[mb, p0:p1, kh * 4:(kh + 1) * 4, :])

            def dma_xd(half, dst):
                # 4KB lines: [32p, 2kt, 1024] pieces, kt-quarter-major
                for kq in range(4):
                    for i in range(4):
                        p0, p1 = i * 32, (i + 1) * 32
                        nc.sync.dma_start(
                            dst[p0:p1, kq * 2:(kq + 1) * 2, :],
                            xd_d[half, p0:p1, kq * 2:(kq + 1) * 2, :])

            # round 1: first-scores critical set (bq first: it gates the
            # prefix Q eviction bias-add)
            bq_sb = consts.tile([128, NT, 1], FP32, tag="bq")
            nc.sync.dma_start(bq_sb[:], bq_d.rearrange("t p o -> p t o"))
            dma_w(wk_sb, wk_d, 4)
            dma_xe(0)
            dma_w(wq_sb, wq_d, 4)
            dma_w(wv_sb, wv_d, 4)
            dma_xd(0, xdh0_sb)
            # round 2: phase-0 front needs
            dma_xe(1)

            # ---- activations kept resident ----
            QT_sb = acts.tile([128, NT, S], BF, tag="QT")    # [head_dim, m]
            KT_sb = acts.tile([128, NT, S], BF, tag="KT")    # [head_dim, j]
            v_sb = acts.tile([128, JT, HPC, KEY + 1], BF, tag="v")  # V' + ones col
            ctxT_sb = acts.tile([128, NT, S], BF, tag="ctxT")

            nc.vector.memset(v_sb[:, :, :, KEY:KEY + 1], 1.0)

            # ================= PSUM =================
            # "ss": 2x[128,1024] (4 banks) scores / out-proj
            # "cc": 4x[128,512] (4 banks) proj chains, V chains, ctx chains
            with (
                tc.tile_pool(name="expp", bufs=34) as expp,
                tc.tile_pool(name="ps", bufs=2, space="PSUM") as ps,
                tc.tile_pool(name="cp", bufs=4, space="PSUM") as cp,
            ):
                # PE warm-up burst (~3.4us at cold clock)
                wt = cp.tile([128, 512], FP32, tag="cc", name="warmup")
                for _ in range(12):
                    nc.tensor.matmul(wt[:], warm_sb[:, 0:128], warm_sb[:],
                                     start=True, stop=True)

                def emit_v_group(g):
                    """V' for key tiles 2g, 2g+1 (a start=True matmul clears its
                    whole PSUM bank, so each chain needs its own bank)."""
                    pv = [cp.tile([128, 512], FP32, tag="cc", name=f"pv_{g}_{d}")
                          for d in range(2)]
                    for kt in range(KT):
                        for d in range(2):
                            j = 2 * g + d
                            nc.tensor.matmul(
                                pv[d][:, 0:NPC],
                                xeT_sb[:, j // 4, kt,
                                       (j % 4) * 128:(j % 4 + 1) * 128],
                                wv_sb[:, kt, :],
                                start=(kt == 0), stop=(kt == KT - 1),
                            )
                    for d in range(2):
                        nc.vector.tensor_copy(
                            v_sb[:, 2 * g + d, :, 0:KEY],
                            pv[d][:, 0:NPC].rearrange("p (h k) -> p h k", h=HPC),
                        )

                # ---- prefix: K0 keys 0-511, V groups 0-1 (fills the
                # DMA wait for xdh0), then Q nt0+nt1 for m 0-1023 ----
                k0c0 = cp.tile([128, 512], FP32, tag="cc", name="k0c0")
                for kt in range(KT):
                    nc.tensor.matmul(
                        k0c0[:], wk_sb[:, kt, 0:128], xeT_sb[:, 0, kt, :],
                        start=(kt == 0), stop=(kt == KT - 1),
                    )
                nc.vector.tensor_copy(KT_sb[:, 0, 0:512], k0c0[:])

                emit_v_group(0)
                emit_v_group(1)

                qc = [cp.tile([128, 512], FP32, tag="cc", name=f"qpre_{i}")
                      for i in range(4)]
                for kt in range(KT):
                    for nt in range(NT):
                        for mc in range(2):
                            nc.tensor.matmul(
                                qc[nt * 2 + mc][:],
                                wq_sb[:, kt, nt * 128:(nt + 1) * 128],
                                xdh0_sb[:, kt, mc * 512:(mc + 1) * 512],
                                start=(kt == 0), stop=(kt == KT - 1),
                            )
                for nt in range(NT):
                    for mc in range(2):
                        nc.vector.tensor_scalar_add(
                            QT_sb[:, nt, mc * 512:(mc + 1) * 512],
                            qc[nt * 2 + mc][:], bq_sb[:, nt, :])

                # ---- remaining DMAs (land during phase 0) ----
                dma_xe(2)
                dma_xe(3)
                # xdT m-half-1: used by the Q dribbles in phases 1/2.
                dma_xd(1, xdh1_sb)
                wo_sb = consts.tile([128, NT, D], BF, tag="wo")
                dma_w(wo_sb, wo_d, 4)

                # ---- dribble emitters (phase-0 / late-phase projections) ----
                def emit_k_chain_part(st, nt, mc, kts):
                    """Part of K-proj chain for KT_sb[:, nt, mc*512:...]."""
                    for kt in kts:
                        nc.tensor.matmul(
                            st[:], wk_sb[:, kt, nt * 128:(nt + 1) * 128],
                            xeT_sb[:, mc, kt, :],
                            start=(kt == 0), stop=(kt == KT - 1),
                        )
                    if kts[-1] == KT - 1:
                        nc.vector.tensor_copy(
                            KT_sb[:, nt, mc * 512:(mc + 1) * 512], st[:])

                def emit_q_chain_part(st, nt, mh, mc, kts):
                    """Part of Q-proj chain: QT_sb[:, nt, mh*1024 + mc*512 ...]
                    reading the resident xdT m-half-1 buffer."""
                    t, _ = st
                    for kt in kts:
                        nc.tensor.matmul(
                            t[:], wq_sb[:, kt, nt * 128:(nt + 1) * 128],
                            xdh1_sb[:, kt, mc * 512:(mc + 1) * 512],
                            start=(kt == 0), stop=(kt == KT - 1),
                        )
                    if kts[-1] == KT - 1:
                        nc.vector.tensor_scalar_add(
                            QT_sb[:, nt, mh * 1024 + mc * 512:
                                  mh * 1024 + (mc + 1) * 512],
                            t[:], bq_sb[:, nt, :])

                # ---- attention phases ----
                rows = [0, KEY]
                order = [(0, 0), (1, 0), (0, 1), (1, 1)]

                def emit_ctx_step(hp, jt, exp_row, ccs):
                    for hh in range(2):
                        h = hp * 2 + hh
                        for q in range(2):
                            nc.tensor.matmul(
                                ccs[hh * 2 + q][0:KEY + 1, :],
                                v_sb[:, jt, h, :],
                                exp_row[hh][:, q * 512:(q + 1) * 512],
                                start=(jt == 0),
                                stop=(jt == JT - 1),
                            )

                def emit_norm(hp, mh, ccs):
                    m0 = mh * 1024
                    for hh in range(2):
                        row = rows[hh]
                        for q in range(2):
                            c = ccs[hh * 2 + q]
                            u = up.tile([KEY + 1, 512], FP32, tag="u")
                            nc.vector.tensor_copy(u[:], c[0:KEY + 1, :])
                            zraw = zp.tile([1, 512], FP32, tag="z")
                            nc.vector.tensor_copy(zraw[:], u[KEY:KEY + 1, :])
                            zb = zbp.tile([KEY, 512], FP32, tag="zb")
                            nc.gpsimd.partition_broadcast(zb[:], zraw[:])
                            zbr = zbp.tile([KEY, 512], FP32, tag="zbr")
                            nc.vector.reciprocal_approx_fast(zbr[:], zb[:])
                            nc.vector.tensor_mul(
                                ctxT_sb[row:row + KEY, hp,
                                        m0 + q * 512:m0 + (q + 1) * 512],
                                u[0:KEY, :],
                                zbr[:],
                            )

                def emit_out(mt, use_cp, dve_only=False, split_dma=False):
                    ot = osb.tile([128, D], BF, tag="ot")
                    if use_cp:
                        pos = [cp.tile([128, 512], FP32, tag="cc",
                                       name=f"po_{mt}_{ec}") for ec in range(2)]
                    else:
                        p1 = ps.tile([128, 1024], FP32, tag="ss", name=f"po_{mt}")
                        pos = [p1[:, 0:512], p1[:, 512:1024]]
                    for dt in range(NT):
                        for ec in range(2):
                            nc.tensor.matmul(
                                pos[ec][0:128, :],
                                ctxT_sb[:, dt, mt * 128:(mt + 1) * 128],
                                wo_sb[:, dt, ec * 512:(ec + 1) * 512],
                                start=(dt == 0),
                                stop=(dt == NT - 1),
                            )
                    for ec in range(2):
                        dst = ot[:, ec * 512:(ec + 1) * 512]
                        if not dve_only and (mt + ec) % 2 == 0:
                            nc.scalar.copy(dst, pos[ec][0:128, :])
                        else:
                            nc.vector.tensor_copy(dst, pos[ec][0:128, :])
                    if split_dma:
                        for ph in range(2):
                            nc.sync.dma_start(
                                o_d[mt * 128 + ph * 64:
                                    mt * 128 + (ph + 1) * 64, :],
                                ot[ph * 64:(ph + 1) * 64, :])
                    else:
                        nc.sync.dma_start(o_d[mt * 128:(mt + 1) * 128, :], ot[:])

                prev = None  # (hp, mh, exp_tiles)
                drib = {}

                for si, (hp, mh) in enumerate(order):
                    m0 = mh * 1024
                    last = si == len(order) - 1
                    cur_cc = None
                    prev_cc = None
                    if prev is not None:
                        prev_cc = [cp.tile([128, 512], FP32, tag="cc",
                                           name=f"cc_{si}_{i}") for i in range(4)]
                    cur_exps = []
                    for jt in range(JT):
                        exp_row = []
                        for hh in range(2):
                            row = rows[hh]
                            ss = ps.tile([128, 1024], FP32, tag="ss")
                            for q in range(2):
                                nc.tensor.matmul(
                                    ss[:, q * 512:(q + 1) * 512],
                                    KT_sb[row:row + KEY, hp, jt * 128:(jt + 1) * 128],
                                    QT_sb[row:row + KEY, hp,
                                          m0 + q * 512:m0 + (q + 1) * 512],
                                    start=True, stop=True,
                                )
                            et = expp.tile([128, 1024], BF, tag="exp")
                            nc.scalar.activation(et[:], ss[:], AF.Exp, scale=0.125)
                            exp_row.append(et)
                        cur_exps.append(exp_row)

                        if si == 1:
                            # ctx0: 1x jt 0-7, 2x jt 8-11; norm0 at jt 12;
                            # jt 12-15 dribble BOTH Q projections for m-half 1
                            # (4 chains in the 4 freed ctx banks).
                            if jt < 8:
                                emit_ctx_step(prev[0], jt, prev[2][jt], prev_cc)
                            elif jt < 12:
                                for j2 in ((jt - 8) * 2 + 8, (jt - 8) * 2 + 9):
                                    emit_ctx_step(prev[0], j2, prev[2][j2], prev_cc)
                            elif jt == 12:
                                emit_norm(prev[0], prev[1], prev_cc)
                                drib["q"] = [
                                    (cp.tile([128, 512], FP32, tag="cc",
                                             name=f"qd_{nt}_{mc}"), nt, mc)
                                    for nt in range(2) for mc in range(2)]
                            if jt >= 12:
                                kts = [2 * (jt - 12), 2 * (jt - 12) + 1]
                                for t, nt, mc in drib["q"]:
                                    emit_q_chain_part((t, None), nt, 1, mc, kts)
                        elif si == 2:
                            # ctx1 at 2x in the front half; norm1 at jt 8;
                            # out-proj mts 0-6 (m-half 0) fill the ACT-bound
                            # back half using the freed ctx banks.
                            if jt < 8:
                                for j2 in (jt * 2, jt * 2 + 1):
                                    emit_ctx_step(prev[0], j2, prev[2][j2], prev_cc)
                                if jt == 7:
                                    emit_norm(prev[0], prev[1], prev_cc)
                            elif jt >= 9:
                                emit_out(jt - 9, True, dve_only=True,
                                         split_dma=True)
                        elif last:
                            # front: ctx(prev) at 2x; norm(prev) at jt 8;
                            # back: own ctx at 2x.
                            if jt < 8:
                                for j2 in (jt * 2, jt * 2 + 1):
                                    emit_ctx_step(prev[0], j2, prev[2][j2], prev_cc)
                                if jt == 7:
                                    emit_norm(prev[0], prev[1], prev_cc)
                            else:
                                if jt == 8:
                                    cur_cc = [cp.tile([128, 512], FP32, tag="cc",
                                                      name=f"cc_last_{i}")
                                              for i in range(4)]
                                for j2 in ((jt - 8) * 2, (jt - 8) * 2 + 1):
                                    emit_ctx_step(hp, j2, cur_exps[j2], cur_cc)
                        if si == 0:
                            # K0 chains 1-3 just ahead of their scores;
                            # V groups jt 0-7; K1 chains jt 8-15.
                            if jt in (2, 3, 4, 5, 6, 7):
                                mc0 = jt // 2  # c1@jt2-3, c2@jt4-5, c3@jt6-7
                                if jt % 2 == 0:
                                    drib["k0"] = cp.tile([128, 512], FP32,
                                                         tag="cc", name=f"k0c{mc0}")
                                emit_k_chain_part(drib["k0"], 0, mc0,
                                                  [4 * (jt % 2) + i for i in range(4)])
                            if 2 <= jt < 8:
                                emit_v_group(jt)
                            if jt >= 8:
                                mc = (jt - 8) // 2
                                if (jt - 8) % 2 == 0:
                                    drib["k1"] = cp.tile([128, 512], FP32,
                                                         tag="cc", name=f"k1c{mc}")
                                emit_k_chain_part(drib["k1"], 1, mc,
                                                  [4 * ((jt - 8) % 2) + i
                                                   for i in range(4)])
                    prev = (hp, mh, cur_exps)

                # ================= output projection tail =================
                # mts 0-6 ran inside phase 2; mt 7 precedes norm3 so the PE
                # streams while the final normalization runs on DVE/GpSimd.
                emit_out(7, False, split_dma=True)
                emit_norm(prev[0], prev[1], cur_cc)
                for mt in range(8, S // 128):
                    # cp tiles only after norm3 has consumed the ctx chains
                    emit_out(mt, mt % 2 == 1, split_dma=True)

    nc.compile()
    return nc


def _get_nc():
    global _NC
    if _NC is None:
        _NC = _build_nc()
    return _NC


def _maybe_register_ntff_hook():
    """Optional: register the axon NTFF profile hook so BASS_TRACE=1 yields
    HW exec times. No-op if unavailable (e.g. the grading environment)."""
    if "antenv.axon_hooks" in sys.modules:
        return
    try:
        import types

        if "/root/.axon_site" not in sys.path and os.path.isdir("/root/.axon_site"):
            sys.path.append("/root/.axon_site")
        from trn_agent_boot.trn_boot import _ntff_profile_via_ctypes

        hook = _ntff_profile_via_ctypes("/opt/axon/libaxon_pjrt.so")
        mod = types.ModuleType("antenv.axon_hooks")
        mod.get_axon_ntff_profile_hook = lambda: hook
        mod.set_axon_ntff_profile_hook = lambda h: None
        sys.modules["antenv.axon_hooks"] = mod
    except Exception:
        pass


def kernel(decoder_output, encoder_output, wq, bq, wk, bk, wv, bv, wo, bo):
    from concourse.bass_utils import run_bass_kernel_spmd

    global LAST_RESULTS

    decoder_output = np.asarray(decoder_output, dtype=np.float32)
    encoder_output = np.asarray(encoder_output, dtype=np.float32)
    wq = np.asarray(wq, dtype=np.float32)
    wk = np.asarray(wk, dtype=np.float32)
    wv = np.asarray(wv, dtype=np.float32)
    wo = np.asarray(wo, dtype=np.float32)
    bq = np.asarray(bq, dtype=np.float32)
    bv = np.asarray(bv, dtype=np.float32)
    bo = np.asarray(bo, dtype=np.float32)
    # bk is softmax-invariant (adds a per-query constant to every logit).

    if os.environ.get("BASS_TRACE"):
        _maybe_register_ntff_hook()

    nc = _get_nc()

    # Pre-tiled host layouts (4-16KB contiguous DMA lines):
    #   xd: [half, p, kt, 1024]   xe: [mb, p, kt, 512]
    #   wq/wk/wv: [p, kt, 256]    wo: [p, nt, 1024]
    xT = {}
    for b in range(B):
        xdT = decoder_output[b].T.astype(BF16)   # [D, S]
        xeT = encoder_output[b].T.astype(BF16)
        xT[("d", b)] = np.ascontiguousarray(
            xdT.reshape(KT, 128, 2, 1024).transpose(2, 1, 0, 3))
        xT[("e", b)] = np.ascontiguousarray(
            xeT.reshape(KT, 128, 4, 512).transpose(2, 1, 0, 3))

    def tile_w(w):  # [D, NPC] -> [128, KT, NPC]
        return np.ascontiguousarray(
            w.astype(BF16).reshape(KT, 128, NPC).transpose(1, 0, 2))

    in_maps = []
    for c in range(8):
        b, hg = c // 4, c % 4
        sl = slice(hg * NPC, (hg + 1) * NPC)
        in_maps.append({
            "xd": xT[("d", b)],
            "xe": xT[("e", b)],
            "wq": tile_w(wq[:, sl]),
            "wk": tile_w(wk[:, sl]),
            "wv": tile_w(wv[:, sl]),
            "wo": np.ascontiguousarray(
                wo[sl, :].astype(BF16).reshape(NT, 128, D).transpose(1, 0, 2)),
            "bq": bq[sl].reshape(NT, 128, 1),
        })

    res = run_bass_kernel_spmd(nc, in_maps, core_ids=list(range(8)))
    LAST_RESULTS = res

    correction = (bv @ wo + bo).astype(np.float32)  # probs sum to 1
    out = np.zeros((B, S, D), dtype=np.float32)
    for c in range(8):
        out[c // 4] += res.results[c]["o"].astype(np.float32)
    out += correction[None, None, :]
    return out


# revision 34
# speedup vs baseline: 1.0276x; 1.0276x over previous
"""Self-contained Trainium2 Bass kernel for 16-head cross-attention MHA.

Problem: B=2, SQ=SK=2048, D=1024, H=16, key_size=64 (fp32 in/out).

Sharding (8 cores): data-parallel over batch (2) x tensor-parallel over
head groups (4 heads per core). Each core computes its 4 heads'
Q/K/V projections (column slices of wq/wk/wv), attention, and a partial
output projection (row slice of wo). Host sums the 4 partial outputs per
batch (bf16 partials, fp32 accumulate) and adds the (bv @ wo + bo)
correction (probs sum to 1, so bv contributes exactly bv @ wo; bk
cancels in softmax).

Per-core pipeline (bf16 matmuls, fp32 PSUM). The kernel is co-bound:
PE matmul ~200us busy, ScalarE exp effectively ~166us (128 EXP
instructions at (1024+284)/1.2GHz each plus ~190ns NX dispatch gap;
PSUM's 8 banks cap the exp tile at [128,1024] with 2-deep ping-pong).
The schedule keeps both engines dense:

  1. PE warm-up matmuls on a memset tile release the HAM clock gate
     (cold PE runs at 1.2 GHz for ~3.4us) while the first DMAs land.
  2. All inputs are host-pre-tiled so every DMA moves 4KB-contiguous
     lines, split into ~128KB partition-sliced pieces across the 16
     queues (2KB-burst DMA: small lines are descriptor-rate bound).
     Priority order: bq, wk, xeT keys 0-511, wq, wv, xdT m-half-0;
     the rest stream during phase 0. xdT halves are kept resident.
  3. Prefix computes only K^T keys 0-511, V' key tiles 0-3, and Q^T
     (both head pairs) for m 0-1023 -- the first score matmul issues
     after ~4.5MB instead of 10.5MB.
  4. Phase order (head_pair, m_half) = (0,0),(1,0),(0,1),(1,1).
     Scores^T tiles ([key_pos, query], contraction = head_dim) are
     exp'd by ScalarE from [128,1024] PSUM (scale=1/8 fused, no
     max-subtraction: scores ~ N(0,1)) into bf16 SBUF. Phase 0
     dribbles the remaining K0 chains just ahead of their scores,
     V' groups 2-7 (ones column carries the softmax denominator Z),
     and K1. ctx for phase i runs inside phase i+1 (1x jt 0-7, 2x jt
     8-11, normalization at jt 12) freeing the 4 ctx PSUM banks for
     the late Q-projection dribbles (m-half-1 in phases 1/2). Phase 3
     runs ctx(phase2) at 2x in its front half and its own ctx at 2x
     in the back half (start=True clears a whole PSUM bank, so every
     accumulation chain owns its banks).
  5. Normalization: U' evicted to SBUF, Z row partition-broadcast via
     GpSimd, DVE reciprocal + multiply into ctx^T bf16.
  6. Tail: out-projection mts 0-5 are emitted before the final norm
     (ctxT dependency tracking is coarse); evictions alternate
     ScalarE/DVE; partial output leaves as bf16.

Measured on 8 axon-tunneled trn2 cores: ~241us mean (239.7-242.4 over
4 runs), rel err 4.9e-3 (bf16-dominated; bf16 output adds ~0.1%).
"""

import os
import sys

for _p in ("/opt/trn_rl_repo", "/root/.axon_site/_ro/trn_rl_repo"):
    if os.path.isdir(_p) and _p not in sys.path:
        sys.path.insert(0, _p)

import numpy as np
import ml_dtypes

BF16 = ml_dtypes.bfloat16

B = 2
S = 2048          # SQ == SK
D = 1024
H = 16
KEY = 64
HPC = 4           # heads per core
NPC = HPC * KEY   # 256 per-core slice of D
KT = D // 128     # 8 contraction tiles for projections
NT = NPC // 128   # 2 head-pair tiles
MC = S // 512     # 4 m-chunks of 512
JT = S // 128     # 16 key tiles

_NC = None
LAST_RESULTS = None  # BassKernelResults of the most recent run (for test.py)


def _build_nc():
    import concourse.tile as tile
    from concourse import bacc, mybir

    FP32 = mybir.dt.float32
    BF = mybir.dt.bfloat16
    AF = mybir.ActivationFunctionType

    nc = bacc.Bacc("TRN2", target_bir_lowering=False, debug=False, num_devices=8)

    # Host pre-tiles every input so each DMA moves >=4KB-contiguous lines
    # (1KB lines are descriptor-rate bound: ~100ns per line per queue).
    xd_d = nc.dram_tensor("xd", [2, 128, KT, 1024], BF, kind="ExternalInput").ap()
    xe_d = nc.dram_tensor("xe", [4, 128, KT, 512], BF, kind="ExternalInput").ap()
    wq_d = nc.dram_tensor("wq", [128, KT, NPC], BF, kind="ExternalInput").ap()
    wk_d = nc.dram_tensor("wk", [128, KT, NPC], BF, kind="ExternalInput").ap()
    wv_d = nc.dram_tensor("wv", [128, KT, NPC], BF, kind="ExternalInput").ap()
    wo_d = nc.dram_tensor("wo", [128, NT, D], BF, kind="ExternalInput").ap()
    bq_d = nc.dram_tensor("bq", [NT, 128, 1], FP32, kind="ExternalInput").ap()
    o_d = nc.dram_tensor("o", [S, D], BF, kind="ExternalOutput").ap()

    with tile.TileContext(nc) as tc:
        with (
            tc.tile_pool(name="consts", bufs=1) as consts,
            tc.tile_pool(name="acts", bufs=1) as acts,
            tc.tile_pool(name="zp", bufs=2) as zp,
            tc.tile_pool(name="up", bufs=4) as up,
            tc.tile_pool(name="zbp", bufs=2) as zbp,
            tc.tile_pool(name="osb", bufs=3) as osb,
        ):
            # ---- PE warm-up (HAM clock gate): no-DMA dummy matmuls ----
            warm_sb = consts.tile([128, 512], BF, tag="warm")
            nc.vector.memset(warm_sb[:], 0.0)

            # ---- priority DMAs: partition-sliced pieces across queues ----
            wk_sb = consts.tile([128, KT, NPC], BF, tag="wk")
            wq_sb = consts.tile([128, KT, NPC], BF, tag="wq")
            wv_sb = consts.tile([128, KT, NPC], BF, tag="wv")
            xeT_sb = acts.tile([128, 4, KT, 512], BF, tag="xeT")
            xdh0_sb = acts.tile([128, KT, 1024], BF, tag="xdh0")
            xdh1_sb = acts.tile([128, KT, 1024], BF, tag="xdh1")

            def dma_w(dst, src, pieces, cols=slice(None)):
                ps_sz = 128 // pieces
                for i in range(pieces):
                    p0, p1 = i * ps_sz, (i + 1) * ps_sz
                    nc.sync.dma_start(dst[p0:p1, :, cols], src[p0:p1, :, cols])

            def dma_xe(mb):
                # 4KB lines: [32p, 4kt, 512] pieces, kt-half-major
                for kh in range(2):
                    for i in range(4):
                        p0, p1 = i * 32, (i + 1) * 32
                        nc.sync.dma_start(
                            xeT_sb[p0:p1, mb, kh * 4:(kh + 1) * 4, :],
                            xe_d[mb, p0:p1, kh * 4:(kh + 1) * 4, :])

            def dma_xd(half, dst):
                # 4KB lines: [32p, 2kt, 1024] pieces, kt-quarter-major
                for kq in range(4):
                    for i in range(4):
                        p0, p1 = i * 32, (i + 1) * 32
                        nc.sync.dma_start(
                            dst[p0:p1, kq * 2:(kq + 1) * 2, :],
                            xd_d[half, p0:p1, kq * 2:(kq + 1) * 2, :])

            # round 1: first-scores critical set (bq first: it gates the
            # prefix Q eviction bias-add)
            bq_sb = consts.tile([128, NT, 1], FP32, tag="bq")
            nc.sync.dma_start(bq_sb[:], bq_d.rearrange("t p o -> p t o"))
            dma_w(wk_sb, wk_d, 4)
            dma_xe(0)
            dma_w(wq_sb, wq_d, 4)
            dma_w(wv_sb, wv_d, 4)
            dma_xd(0, xdh0_sb)
            # round 2: phase-0 front needs
            dma_xe(1)

            # ---- activations kept resident ----
            QT_sb = acts.tile([128, NT, S], BF, tag="QT")    # [head_dim, m]
            KT_sb = acts.tile([128, NT, S], BF, tag="KT")    # [head_dim, j]
            v_sb = acts.tile([128, JT, HPC, KEY + 1], BF, tag="v")  # V' + ones col
            ctxT_sb = acts.tile([128, NT, S], BF, tag="ctxT")

            nc.vector.memset(v_sb[:, :, :, KEY:KEY + 1], 1.0)

            # ================= PSUM =================
            # "ss": 2x[128,1024] (4 banks) scores / out-proj
            # "cc": 4x[128,512] (4 banks) proj chains, V chains, ctx chains
            with (
                tc.tile_pool(name="expp", bufs=34) as expp,
                tc.tile_pool(name="ps", bufs=2, space="PSUM") as ps,
                tc.tile_pool(name="cp", bufs=4, space="PSUM") as cp,
            ):
                # PE warm-up burst (~3.4us at cold clock)
                wt = cp.tile([128, 512], FP32, tag="cc", name="warmup")
                for _ in range(12):
                    nc.tensor.matmul(wt[:], warm_sb[:, 0:128], warm_sb[:],
                                     start=True, stop=True)

                def emit_v_group(g):
                    """V' for key tiles 2g, 2g+1 (a start=True matmul clears its
                    whole PSUM bank, so each chain needs its own bank)."""
                    pv = [cp.tile([128, 512], FP32, tag="cc", name=f"pv_{g}_{d}")
                          for d in range(2)]
                    for kt in range(KT):
                        for d in range(2):
                            j = 2 * g + d
                            nc.tensor.matmul(
                                pv[d][:, 0:NPC],
                                xeT_sb[:, j // 4, kt,
                                       (j % 4) * 128:(j % 4 + 1) * 128],
                                wv_sb[:, kt, :],
                                start=(kt == 0), stop=(kt == KT - 1),
                            )
                    for d in range(2):
                        nc.vector.tensor_copy(
                            v_sb[:, 2 * g + d, :, 0:KEY],
                            pv[d][:, 0:NPC].rearrange("p (h k) -> p h k", h=HPC),
                        )

                # ---- prefix: K0 keys 0-511, V groups 0-1 (fills the
                # DMA wait for xdh0), then Q nt0+nt1 for m 0-1023 ----
                k0c0 = cp.tile([128, 512], FP32, tag="cc", name="k0c0")
                for kt in range(KT):
                    nc.tensor.matmul(
                        k0c0[:], wk_sb[:, kt, 0:128], xeT_sb[:, 0, kt, :],
                        start=(kt == 0), stop=(kt == KT - 1),
                    )
                nc.vector.tensor_copy(KT_sb[:, 0, 0:512], k0c0[:])

                emit_v_group(0)
                emit_v_group(1)

                qc = [cp.tile([128, 512], FP32, tag="cc", name=f"qpre_{i}")
                      for i in range(4)]
                for kt in range(KT):
                    for nt in range(NT):
                        for mc in range(2):
                            nc.tensor.matmul(
                                qc[nt * 2 + mc][:],
                                wq_sb[:, kt, nt * 128:(nt + 1) * 128],
                                xdh0_sb[:, kt, mc * 512:(mc + 1) * 512],
                                start=(kt == 0), stop=(kt == KT - 1),
                            )
                for nt in range(NT):
                    for mc in range(2):
                        nc.vector.tensor_scalar_add(
                            QT_sb[:, nt, mc * 512:(mc + 1) * 512],
                            qc[nt * 2 + mc][:], bq_sb[:, nt, :])

                # ---- remaining DMAs (land during phase 0) ----
                dma_xe(2)
                dma_xe(3)
                # xdT m-half-1: used by the Q dribbles in phases 1/2.
                dma_xd(1, xdh1_sb)
                wo_sb = consts.tile([128, NT, D], BF, tag="wo")
                dma_w(wo_sb, wo_d, 4)

                # ---- dribble emitters (phase-0 / late-phase projections) ----
                def emit_k_chain_part(st, nt, mc, kts):
                    """Part of K-proj chain for KT_sb[:, nt, mc*512:...]."""
                    for kt in kts:
                        nc.tensor.matmul(
                            st[:], wk_sb[:, kt, nt * 128:(nt + 1) * 128],
                            xeT_sb[:, mc, kt, :],
                            start=(kt == 0), stop=(kt == KT - 1),
                        )
                    if kts[-1] == KT - 1:
                        nc.vector.tensor_copy(
                            KT_sb[:, nt, mc * 512:(mc + 1) * 512], st[:])

                def emit_q_chain_part(st, nt, mh, mc, kts):
                    """Part of Q-proj chain: QT_sb[:, nt, mh*1024 + mc*512 ...]
                    reading the resident xdT m-half-1 buffer."""
                    t, _ = st
                    for kt in kts:
                        nc.tensor.matmul(
                            t[:], wq_sb[:, kt, nt * 128:(nt + 1) * 128],
                            xdh1_sb[:, kt, mc * 512:(mc + 1) * 512],
                            start=(kt == 0), stop=(kt == KT - 1),
                        )
                    if kts[-1] == KT - 1:
                        nc.vector.tensor_scalar_add(
                            QT_sb[:, nt, mh * 1024 + mc * 512:
                                  mh * 1024 + (mc + 1) * 512],
                            t[:], bq_sb[:, nt, :])

                # ---- attention phases ----
                rows = [0, KEY]
                order = [(0, 0), (1, 0), (0, 1), (1, 1)]

                def emit_ctx_step(hp, jt, exp_row, ccs):
                    for hh in range(2):
                        h = hp * 2 + hh
                        for q in range(2):
                            nc.tensor.matmul(
                                ccs[hh * 2 + q][0:KEY + 1, :],
                                v_sb[:, jt, h, :],
                                exp_row[hh][:, q * 512:(q + 1) * 512],
                                start=(jt == 0),
                                stop=(jt == JT - 1),
                            )

                def emit_norm(hp, mh, ccs):
                    m0 = mh * 1024
                    for hh in range(2):
                        row = rows[hh]
                        for q in range(2):
                            c = ccs[hh * 2 + q]
                            u = up.tile([KEY + 1, 512], FP32, tag="u")
                            nc.vector.tensor_copy(u[:], c[0:KEY + 1, :])
                            zraw = zp.tile([1, 512], FP32, tag="z")
                            nc.vector.tensor_copy(zraw[:], u[KEY:KEY + 1, :])
                            zb = zbp.tile([KEY, 512], FP32, tag="zb")
                            nc.gpsimd.partition_broadcast(zb[:], zraw[:])
                            zbr = zbp.tile([KEY, 512], FP32, tag="zbr")
                            nc.vector.reciprocal_approx_fast(zbr[:], zb[:])
                            nc.vector.tensor_mul(
                                ctxT_sb[row:row + KEY, hp,
                                        m0 + q * 512:m0 + (q + 1) * 512],
                                u[0:KEY, :],
                                zbr[:],
                            )

                def emit_out(mt, use_cp, dve_only=False, split_dma=False):
                    ot = osb.tile([128, D], BF, tag="ot")
                    if use_cp:
                        pos = [cp.tile([128, 512], FP32, tag="cc",
                                       name=f"po_{mt}_{ec}") for ec in range(2)]
                    else:
                        p1 = ps.tile([128, 1024], FP32, tag="ss", name=f"po_{mt}")
                        pos = [p1[:, 0:512], p1[:, 512:1024]]
                    for dt in range(NT):
                        for ec in range(2):
                            nc.tensor.matmul(
                                pos[ec][0:128, :],
                                ctxT_sb[:, dt, mt * 128:(mt + 1) * 128],
                                wo_sb[:, dt, ec * 512:(ec + 1) * 512],
                                start=(dt == 0),
                                stop=(dt == NT - 1),
                            )
                    for ec in range(2):
                        dst = ot[:, ec * 512:(ec + 1) * 512]
                        if not dve_only and (mt + ec) % 2 == 0:
                            nc.scalar.copy(dst, pos[ec][0:128, :])
                        else:
                            nc.vector.tensor_copy(dst, pos[ec][0:128, :])
                    if split_dma:
                        for ph in range(2):
                            nc.sync.dma_start(
                                o_d[mt * 128 + ph * 64:
                                    mt * 128 + (ph + 1) * 64, :],
                                ot[ph * 64:(ph + 1) * 64, :])
                    else:
                        nc.sync.dma_start(o_d[mt * 128:(mt + 1) * 128, :], ot[:])

                prev = None  # (hp, mh, exp_tiles)
                drib = {}

                for si, (hp, mh) in enumerate(order):
                    m0 = mh * 1024
                    last = si == len(order) - 1
                    cur_cc = None
                    prev_cc = None
                    if prev is not None:
                        prev_cc = [cp.tile([128, 512], FP32, tag="cc",
                                           name=f"cc_{si}_{i}") for i in range(4)]
                    cur_exps = []
                    for jt in range(JT):
                        exp_row = []
                        for hh in range(2):
                            row = rows[hh]
                            ss = ps.tile([128, 1024], FP32, tag="ss")
                            for q in range(2):
                                nc.tensor.matmul(
                                    ss[:, q * 512:(q + 1) * 512],
                                    KT_sb[row:row + KEY, hp, jt * 128:(jt + 1) * 128],
                                    QT_sb[row:row + KEY, hp,
                                          m0 + q * 512:m0 + (q + 1) * 512],
                                    start=True, stop=True,
                                )
                            et = expp.tile([128, 1024], BF, tag="exp")
                            nc.scalar.activation(et[:], ss[:], AF.Exp, scale=0.125)
                            exp_row.append(et)
                        cur_exps.append(exp_row)

                        if si in (1, 2):
                            # ctx(prev): 1x jt 0-7, 2x jt 8-11; norm at jt 12;
                            # jt 12-15 dribble Q projections for m-half 1.
                            if jt < 8:
                                emit_ctx_step(prev[0], jt, prev[2][jt], prev_cc)
                            elif jt < 12:
                                for j2 in ((jt - 8) * 2 + 8, (jt - 8) * 2 + 9):
                                    emit_ctx_step(prev[0], j2, prev[2][j2], prev_cc)
                            elif jt == 12:
                                emit_norm(prev[0], prev[1], prev_cc)
                                nt = si - 1  # ph1 -> Q nt0 mh1, ph2 -> Q nt1 mh1
                                t = cp.tile([128, 512], FP32, tag="cc",
                                            name=f"qd{si}_a")
                                t2 = cp.tile([128, 512], FP32, tag="cc",
                                             name=f"qd{si}_b")
                                drib["q"] = ((t, None), (t2, None), nt)
                            if jt >= 12:
                                kts = [2 * (jt - 12), 2 * (jt - 12) + 1]
                                sta, stb, nt = drib["q"]
                                emit_q_chain_part(sta, nt, 1, 0, kts)
                                emit_q_chain_part(stb, nt, 1, 1, kts)
                        elif last:
                            # front: ctx(prev) at 2x; norm(prev) at jt 8;
                            # back: own ctx at 2x.
                            if jt < 8:
                                for j2 in (jt * 2, jt * 2 + 1):
                                    emit_ctx_step(prev[0], j2, prev[2][j2], prev_cc)
                                if jt == 7:
                                    emit_norm(prev[0], prev[1], prev_cc)
                            else:
                                if jt == 8:
                                    cur_cc = [cp.tile([128, 512], FP32, tag="cc",
                                                      name=f"cc_last_{i}")
                                              for i in range(4)]
                                for j2 in ((jt - 8) * 2, (jt - 8) * 2 + 1):
                                    emit_ctx_step(hp, j2, cur_exps[j2], cur_cc)
                        if si == 0:
                            # K0 chains 1-3 just ahead of their scores;
                            # V groups jt 0-7; K1 chains jt 8-15.
                            if jt in (2, 3, 4, 5, 6, 7):
                                mc0 = jt // 2  # c1@jt2-3, c2@jt4-5, c3@jt6-7
                                if jt % 2 == 0:
                                    drib["k0"] = cp.tile([128, 512], FP32,
                                                         tag="cc", name=f"k0c{mc0}")
                                emit_k_chain_part(drib["k0"], 0, mc0,
                                                  [4 * (jt % 2) + i for i in range(4)])
                            if 2 <= jt < 8:
                                emit_v_group(jt)
                            if jt >= 8:
                                mc = (jt - 8) // 2
                                if (jt - 8) % 2 == 0:
                                    drib["k1"] = cp.tile([128, 512], FP32,
                                                         tag="cc", name=f"k1c{mc}")
                                emit_k_chain_part(drib["k1"], 1, mc,
                                                  [4 * ((jt - 8) % 2) + i
                                                   for i in range(4)])
                    prev = (hp, mh, cur_exps)

                # ================= output projection tail =================
                # mts 0-5 precede norm3 so the PE streams while the final
                # normalization runs on DVE/GpSimd.
                for mt in range(6):
                    emit_out(mt, False)
                emit_norm(prev[0], prev[1], cur_cc)
                for mt in range(6, S // 128):
                    # cp tiles only after norm3 has consumed the ctx chains
                    emit_out(mt, mt % 2 == 1)

    nc.compile()
    return nc


def _get_nc():
    global _NC
    if _NC is None:
        _NC = _build_nc()
    return _NC


def _maybe_register_ntff_hook():
    """Optional: register the axon NTFF profile hook so BASS_TRACE=1 yields
    HW exec times. No-op if unavailable (e.g. the grading environment)."""
    if "antenv.axon_hooks" in sys.modules:
        return
    try:
        import types

        if "/root/.axon_site" not in sys.path and os.path.isdir("/root/.axon_site"):
            sys.path.append("/root/.axon_site")
        from trn_agent_boot.trn_boot import _ntff_profile_via_ctypes

        hook = _ntff_profile_via_ctypes("/opt/axon/libaxon_pjrt.so")
        mod = types.ModuleType("antenv.axon_hooks")
        mod.get_axon_ntff_profile_hook = lambda: hook
        mod.set_axon_ntff_profile_hook = lambda h: None
        sys.modules["antenv.axon_hooks"] = mod
    except Exception:
        pass


def kernel(decoder_output, encoder_output, wq, bq, wk, bk, wv, bv, wo, bo):
    from concourse.bass_utils import run_bass_kernel_spmd

    global LAST_RESULTS

    decoder_output = np.asarray(decoder_output, dtype=np.float32)
    encoder_output = np.asarray(encoder_output, dtype=np.float32)
    wq = np.asarray(wq, dtype=np.float32)
    wk = np.asarray(wk, dtype=np.float32)
    wv = np.asarray(wv, dtype=np.float32)
    wo = np.asarray(wo, dtype=np.float32)
    bq = np.asarray(bq, dtype=np.float32)
    bv = np.asarray(bv, dtype=np.float32)
    bo = np.asarray(bo, dtype=np.float32)
    # bk is softmax-invariant (adds a per-query constant to every logit).

    if os.environ.get("BASS_TRACE"):
        _maybe_register_ntff_hook()

    nc = _get_nc()

    # Pre-tiled host layouts (4-16KB contiguous DMA lines):
    #   xd: [half, p, kt, 1024]   xe: [mb, p, kt, 512]
    #   wq/wk/wv: [p, kt, 256]    wo: [p, nt, 1024]
    xT = {}
    for b in range(B):
        xdT = decoder_output[b].T.astype(BF16)   # [D, S]
        xeT = encoder_output[b].T.astype(BF16)
        xT[("d", b)] = np.ascontiguousarray(
            xdT.reshape(KT, 128, 2, 1024).transpose(2, 1, 0, 3))
        xT[("e", b)] = np.ascontiguousarray(
            xeT.reshape(KT, 128, 4, 512).transpose(2, 1, 0, 3))

    def tile_w(w):  # [D, NPC] -> [128, KT, NPC]
        return np.ascontiguousarray(
            w.astype(BF16).reshape(KT, 128, NPC).transpose(1, 0, 2))

    in_maps = []
    for c in range(8):
        b, hg = c // 4, c % 4
        sl = slice(hg * NPC, (hg + 1) * NPC)
        in_maps.append({
            "xd": xT[("d", b)],
            "xe": xT[("e", b)],
            "wq": tile_w(wq[:, sl]),
            "wk": tile_w(wk[:, sl]),
            "wv": tile_w(wv[:, sl]),
            "wo": np.ascontiguousarray(
                wo[sl, :].astype(BF16).reshape(NT, 128, D).transpose(1, 0, 2)),
            "bq": bq[sl].reshape(NT, 128, 1),
        })

    res = run_bass_kernel_spmd(nc, in_maps, core_ids=list(range(8)))
    LAST_RESULTS = res

    correction = (bv @ wo + bo).astype(np.float32)  # probs sum to 1
    out = np.zeros((B, S, D), dtype=np.float32)
    for c in range(8):
        out[c // 4] += res.results[c]["o"].astype(np.float32)
    out += correction[None, None, :]
    return out
